# revision 63
# baseline (speedup 1.0000x reference)
import sys, os

sys.path.insert(0, "/opt/trn_rl_repo")
sys.path.insert(0, "/root/.axon_site")
import numpy as np

DIM = 2048
DH = 64
H = 16
HKV = 4
G = H // HKV
RANK = 8
S = 2048
NCORES = 8
NB = S // 128   # 16 q-blocks of 128 rows
NSPAN = 4       # 4 spans of 512 over S
SPAN = 512
ND = DIM // 128  # 16 D-tiles

_CACHE = {}


def _deint_perm():
    # even dims 0,2,..62 -> rows 0..31 ; odd dims -> rows 32..63
    p = np.zeros(DH, np.int64)
    for i in range(DH // 2):
        p[i] = 2 * i
        p[32 + i] = 2 * i + 1
    return p


def _prep(inputs):
    """Host-side prep; returns per-core input maps (uniform shapes)."""
    f16 = np.float16
    x = np.asarray(inputs["x"], np.float32)[0]          # (S, D)
    xt = np.ascontiguousarray(x.T).astype(f16)          # (D, S)
    perm = _deint_perm()

    wq = np.asarray(inputs["wq"], np.float32)[perm] * 0.125   # (64, D) permuted + scale
    wk = np.asarray(inputs["wk"], np.float32)[perm]
    wv = np.asarray(inputs["wv"], np.float32)
    wq_a = np.asarray(inputs["wq_a"], np.float32)
    wk_a = np.asarray(inputs["wk_a"], np.float32)
    wv_a = np.asarray(inputs["wv_a"], np.float32)
    wq_b = np.asarray(inputs["wq_b"], np.float32).reshape(H, DH, RANK)[:, perm, :]
    wk_b = np.asarray(inputs["wk_b"], np.float32).reshape(HKV, DH, RANK)[:, perm, :]
    wv_b = np.asarray(inputs["wv_b"], np.float32).reshape(HKV, DH, RANK)

    w1t = np.ascontiguousarray(np.concatenate([wk, wv], 0).T).astype(f16)      # (D, 128)
    w2 = np.zeros((48, DIM), np.float32)   # 32-aligned: k_a@0:8, v_a@32:40
    w2[0:8] = wk_a
    w2[32:40] = wv_a
    w2t = np.ascontiguousarray(w2.T).astype(f16)                               # (D, 48)
    wqt = np.ascontiguousarray(np.concatenate([wq, wq_a], 0).T).astype(f16)    # (D, 72)

    def baug(wb, scale, swap):
        nh = wb.shape[0]
        out = np.zeros((nh // 2, 128, 128), np.float32)
        for m in range(nh // 2):
            for hh in range(2):
                h = 2 * m + hh
                for d in range(DH):
                    dd = (d + 32) % DH if swap else d
                    col = 64 * hh + d
                    out[m, dd, col] = 1.0
                    out[m, 64:72, col] = wb[h, dd] * scale
        return out.astype(f16)

    kba = baug(wk_b, 2.0, False)
    kbs = baug(wk_b, 2.0, True)
    qba = baug(wq_b, 0.25, False)
    qbs = baug(wq_b, 0.25, True)

    # v B-proj rhs with interleaved ones-columns:
    # per m chunk [128, 130] = [v(2m) 64 | one | v(2m+1) 64 | one]
    vba2 = np.zeros((128, 2 * 130), np.float32)
    for m in range(2):
        o = m * 130
        for hh in range(2):
            h = 2 * m + hh
            co = o + hh * 65
            for d in range(DH):
                vba2[d, co + d] = 1.0
                vba2[64:72, co + d] = wv_b[h, d] * 2.0
        vba2[96, o + 64] = 1.0
        vba2[96, o + 129] = 1.0
    vba2 = vba2.astype(f16)

    wo = np.asarray(inputs["wo"], np.float32)              # (D, 64)
    wo_share = np.asarray(inputs["wo_share"], np.float32)  # (D, 1024)
    wc = wo_share + np.tile(wo, (1, H))
    wct = np.ascontiguousarray(wc.T).astype(f16)           # (1024, D)

    fc = np.asarray(inputs["freq_cis"], np.float32)        # (S, 32, 2)
    cos = fc[:, :, 0].T                                    # (32, S)
    sin = fc[:, :, 1].T
    crep = np.tile(cos, (4, 1)).astype(f16)                # (128, S)
    sr = np.concatenate([-sin, sin], 0)                    # (64, S)
    srep = np.tile(sr, (2, 1)).astype(f16)                 # (128, S)

    tri = (np.arange(128)[:, None] <= np.arange(128)[None, :]).astype(f16)
    tri4 = np.ascontiguousarray(np.tile(tri, (1, 4)))      # (128, 512)
    ident = np.eye(128, dtype=f16)
    mask4 = np.ascontiguousarray((1.0 - tri4) * np.float16(-30000.0)).astype(f16)

    # pre-rearrange to the exact SBUF image [128, free] so every DMA is a
    # contiguous per-partition copy (strided gathers were ~3x slower)
    def sbimg(a2d, p=128):
        # (d p) f -> p (d f)
        D2, F = a2d.shape
        d = D2 // p
        return np.ascontiguousarray(a2d.reshape(d, p, F).transpose(1, 0, 2).reshape(p, d * F))

    xtsp = np.stack([sbimg(np.ascontiguousarray(xt[:, sp * 512:(sp + 1) * 512]))
                     for sp in range(4)])                      # (4, 128, 8192)
    w1c = sbimg(w1t)
    w2c = sbimg(w2t)
    wqc = sbimg(wqt)
    wcte = np.stack([sbimg(np.ascontiguousarray(wct[:, e * 512:(e + 1) * 512]))
                     for e in range(4)])                       # (4, 128, 4096)
    kbaf = np.ascontiguousarray(kba.transpose(1, 0, 2).reshape(128, 2 * 128))
    kbsf = np.ascontiguousarray(kbs.transpose(1, 0, 2).reshape(128, 2 * 128))
    qbaf = np.ascontiguousarray(qba.transpose(1, 0, 2).reshape(128, 8 * 128))
    qbsf = np.ascontiguousarray(qbs.transpose(1, 0, 2).reshape(128, 8 * 128))

    shared = dict(
        xtsp=xtsp, w1c=w1c, w2c=w2c, wqc=wqc,
        kbaf=kbaf, kbsf=kbsf, qbaf=qbaf, qbsf=qbsf, vba2=vba2,
        wcte=wcte, crep=crep, srep=srep, tri4=tri4,
    )

    per_core = []
    for c in range(NCORES):
        blocks = [c, 15 - c]
        cols = np.concatenate([np.arange(b * 128, (b + 1) * 128) for b in blocks])
        m = dict(shared)
        m.update(
            xqc=sbimg(np.ascontiguousarray(xt[:, cols])),
            crepq=np.ascontiguousarray(crep[:, cols]),
            srepq=np.ascontiguousarray(srep[:, cols]),
        )
        per_core.append(m)
    return per_core


def _build_program():
    import concourse.bass as bass
    import concourse.bacc as bacc
    import concourse.mybir as mybir
    from concourse import tile

    f16 = mybir.dt.float16
    f32 = mybir.dt.float32
    AF = mybir.ActivationFunctionType

    nc = bacc.Bacc("TRN2", target_bir_lowering=False)

    def inp(name, shape, dt=f16):
        return nc.dram_tensor(name, list(shape), dt, kind="ExternalInput")

    xtsp = inp("xtsp", (NSPAN, 128, ND * SPAN))
    w1c = inp("w1c", (128, ND * 128))
    w2c = inp("w2c", (128, ND * 48))
    wqc = inp("wqc", (128, ND * 72))
    kba = inp("kbaf", (128, 2 * 128))
    kbs = inp("kbsf", (128, 2 * 128))
    qba = inp("qbaf", (128, 8 * 128))
    qbs = inp("qbsf", (128, 8 * 128))
    vba2 = inp("vba2", (128, 260))
    wcte = inp("wcte", (4, 128, 8 * 512))
    crep = inp("crep", (128, S))
    srep = inp("srep", (128, S))
    tri4 = inp("tri4", (128, 512))
    xqc = inp("xqc", (128, ND * 256))
    crepq = inp("crepq", (128, 256))
    srepq = inp("srepq", (128, 256))

    yout = nc.dram_tensor("y", [2, 128, DIM], f16, kind="ExternalOutput")
    DBG = bool(os.environ.get("KDBG"))
    if DBG:
        dbg = {
            "cko": nc.dram_tensor("cko", [128, S], f16, kind="ExternalOutput"),
            "cvo": nc.dram_tensor("cvo", [128, S], f16, kind="ExternalOutput"),
            "cqo": nc.dram_tensor("cqo", [128, 256], f16, kind="ExternalOutput"),
            "kTo": nc.dram_tensor("kTo", [64, HKV * S], f16, kind="ExternalOutput"),
            "qTo": nc.dram_tensor("qTo", [64, 2 * H * 128], f16, kind="ExternalOutput"),
            "vsbo": nc.dram_tensor("vsbo", [128, 2 * NB * 130], f16, kind="ExternalOutput"),
            "onormo": nc.dram_tensor("onormo", [128, 8 * 256], f16, kind="ExternalOutput"),
        }

    pid = nc.partition_id()

    with tile.TileContext(nc) as tc:
        with (
            tc.tile_pool(name="const", bufs=1) as constp,
            tc.tile_pool(name="xts", bufs=2) as xtp,
            tc.tile_pool(name="pt", bufs=2) as ptp,
            tc.tile_pool(name="ev", bufs=2) as evp,
            tc.tile_pool(name="big", bufs=3, space="PSUM") as bigp,
            tc.tile_pool(name="pacc", bufs=1, space="PSUM") as paccp,
        ):
            # ---------------- persistent SBUF ----------------
            # DMA queue plan (engine queues serialize; spread + order by need):
            #  scalar: w1s, span1, wte0, wte1
            #  sync:   span0 (2 halves), span2, wte2, wte3
            #  vector: wqs, xqs, span3
            #  gpsimd: w2s, kba/kbs, creps/sreps, vba/qba/qbs, crepq/srepq, tris
            w1s = constp.tile([128, ND * 128], f16, tag="w1s", name="w1s")
            w2s = constp.tile([128, ND * 48], f16, tag="w2s", name="w2s")
            wqs = constp.tile([128, ND * 72], f16, tag="wqs", name="wqs")
            xqs = constp.tile([128, ND * 256], f16, tag="xqs", name="xqs")
            # priority load: the first weight the PE needs
            nc.scalar.dma_start(out=w1s[:], in_=w1c[:])

            kbas = constp.tile([128, 2 * 128], f16, tag="kbas", name="kbas")
            kbss = constp.tile([128, 2 * 128], f16, tag="kbss", name="kbss")
            creps = constp.tile([128, S], f16, tag="creps", name="creps")
            sreps = constp.tile([128, S], f16, tag="sreps", name="sreps")
            vbas = constp.tile([128, 2 * 130], f16, tag="vbas", name="vbas")
            qbas = constp.tile([128, 8 * 128], f16, tag="qbas", name="qbas")
            qbss = constp.tile([128, 8 * 128], f16, tag="qbss", name="qbss")
            crepqs = constp.tile([128, 256], f16, tag="crepqs", name="crepqs")
            srepqs = constp.tile([128, 256], f16, tag="srepqs", name="srepqs")
            tris = constp.tile([128, 512], f16, tag="tris", name="tris")

            wtes = [constp.tile([128, 8 * 512], f16, tag=f"wte{e}", name=f"wte{e}")
                    for e in range(4)]

            def issue_const_dmas():
                # gated behind stage-1 progress so these transfers don't
                # contend with the critical span-0/weight loads; ordered by
                # first use (stage-2 k, q, v consts, mask, out-proj weights)
                nc.gpsimd.dma_start(out=kbas[:], in_=kba[:])
                nc.gpsimd.dma_start(out=kbss[:], in_=kbs[:])
                nc.gpsimd.dma_start(out=vbas[:], in_=vba2[:])
                nc.gpsimd.dma_start(out=crepqs[:], in_=crepq[:])
                nc.gpsimd.dma_start(out=srepqs[:], in_=srepq[:])
                nc.gpsimd.dma_start(out=creps[:], in_=crep[:])
                nc.gpsimd.dma_start(out=sreps[:], in_=srep[:])
                nc.gpsimd.dma_start(out=qbas[:], in_=qba[:])
                nc.gpsimd.dma_start(out=qbss[:], in_=qbs[:])
                nc.gpsimd.dma_start(out=tris[:], in_=tri4[:])
                for e in range(4):
                    nc.gpsimd.dma_start(out=wtes[e][:], in_=wcte[e, :, :])

            ck = constp.tile([128, S], f16, tag="ck", name="ck")
            cv = constp.tile([128, S], f16, tag="cv", name="cv")
            cq = constp.tile([128, 256], f16, tag="cq", name="cq")
            kT = constp.tile([64, HKV * S], f16, tag="kT", name="kT")
            vsb = constp.tile([128, 2 * NB * 130], f16, tag="vsb", name="vsb")
            qT = constp.tile([64, 2 * H * 128], f16, tag="qT", name="qT")
            onorm = constp.tile([128, 8 * 256], f16, tag="onorm", name="onorm")

            # warm-up: keep the PE busy while the first loads land so the
            # HAM clock gate releases (4/8 -> 8/8) before real work starts
            dmy = constp.tile([128, 512], f16, tag="dmy", name="dmy")
            nc.vector.memset(dmy[:], 0.0)
            for _ in range(16):
                pw = bigp.tile([128, 1024], f32, tag="big", name="pw")
                nc.tensor.matmul(pw[:, 0:512], dmy[:, 0:128], dmy[:],
                                 start=True, stop=True)

            ones1 = constp.tile([1, 64], f16, tag="ones1", name="ones1")
            nc.vector.memset(ones1[:], 1.0)
            nc.vector.memset(ck[:], 0.0)
            nc.gpsimd.memset(cv[:], 0.0)
            nc.vector.memset(cq[:], 0.0)
            nc.gpsimd.memset(cv[96:97, :], 1.0)   # ones row for v denominator trick

            # ---------------- stage 1: projections (uniform) ----------------
            # span DMAs: issue all up front (split into halves for earlier
            # compute start), spread across scalar/sync/vector queues.
            span_eng = [nc.sync, nc.scalar, nc.sync, nc.scalar]
            xtas = [None] * NSPAN

            def load_span(sp):
                xta = xtp.tile([128, ND * SPAN], f16, tag="xta", name="xta")
                if sp == 0:
                    # race span 0 in on three queues at once
                    engs = [nc.sync, nc.sync, nc.scalar, nc.gpsimd]
                    for h in range(4):
                        dlo, dhi = h * 4, (h + 1) * 4
                        engs[h].dma_start(out=xta[:, dlo * SPAN:dhi * SPAN],
                                          in_=xtsp[sp, :, dlo * SPAN:dhi * SPAN])
                else:
                    nc.scalar.dma_start(out=xta[:, 0:8 * SPAN], in_=xtsp[sp, :, 0:8 * SPAN])
                    nc.sync.dma_start(out=xta[:, 8 * SPAN:], in_=xtsp[sp, :, 8 * SPAN:])
                xtas[sp] = xta

            def s1_kv(sp):
                xta = xtas[sp]
                pkv = bigp.tile([128, 1024], f32, tag="big", name="pkv")
                for d in range(ND):
                    nc.tensor.matmul(pkv[:, 0:512], w1s[:, d * 128:(d + 1) * 128],
                                     xta[:, d * SPAN:(d + 1) * SPAN],
                                     start=(d == 0), stop=(d == ND - 1))
                for d in range(ND):
                    nc.tensor.matmul(pkv[0:48, 512:1024], w2s[:, d * 48:(d + 1) * 48],
                                     xta[:, d * SPAN:(d + 1) * SPAN],
                                     start=(d == 0), stop=(d == ND - 1))
                sl = slice(sp * SPAN, (sp + 1) * SPAN)
                nc.vector.tensor_copy(ck[0:64, sl], pkv[0:64, 0:512])
                nc.vector.tensor_copy(ck[64:72, sl], pkv[0:8, 512:1024])
                nc.scalar.copy(cv[0:64, sl], pkv[64:128, 0:512])
                nc.vector.tensor_copy(cv[64:72, sl], pkv[32:40, 512:1024])

            load_span(0)
            # rest of the near-term weights, after span 0 is in flight
            nc.scalar.dma_start(out=w2s[:], in_=w2c[:])
            nc.gpsimd.dma_start(out=wqs[:], in_=wqc[:])
            nc.gpsimd.dma_start(out=xqs[:], in_=xqc[:])
            load_span(1)
            s1_kv(0)
            # gate the bulk const loads behind span-0 eviction so their
            # transfers don't steal HBM bandwidth from the critical path
            gatet = constp.tile([1, 1], f16, tag="gatet", name="gatet")
            nc.gpsimd.tensor_copy(gatet[:], ck[0:1, 0:1])
            issue_const_dmas()
            load_span(2)
            s1_kv(1)
            pqt = bigp.tile([128, 1024], f32, tag="big", name="pqt")
            for d in range(ND):
                nc.tensor.matmul(pqt[0:72, 0:256], wqs[:, d * 72:(d + 1) * 72],
                                 xqs[:, d * 256:(d + 1) * 256],
                                 start=(d == 0), stop=(d == ND - 1))
            nc.vector.tensor_copy(cq[0:72, :], pqt[0:72, 0:256])
            # (spans 2/3 compute is interleaved with stage 2 below)

            # ---------------- stage 2: B-projections + rope (uniform) ----------------
            def s2_k(m, sp):
                sl = slice(sp * SPAN, (sp + 1) * SPAN)
                pk = bigp.tile([128, 1024], f32, tag="big", name="pk")
                nc.tensor.matmul(pk[:, 0:512], kbas[:, m * 128:(m + 1) * 128], ck[:, sl],
                                 start=True, stop=True)
                nc.tensor.matmul(pk[:, 512:1024], kbss[:, m * 128:(m + 1) * 128], ck[:, sl],
                                 start=True, stop=True)
                pks_s = evp.tile([128, 512], f16, tag="pks_s", name="pks_s")
                nc.scalar.copy(pks_s[:], pk[:, 512:1024])
                t1 = evp.tile([128, 512], f16, tag="t1", name="t1")
                t2 = evp.tile([128, 512], f16, tag="t2", name="t2")
                nc.vector.tensor_mul(t1[:], pk[:, 0:512], creps[:, sl])
                nc.vector.tensor_mul(t2[:], pks_s[:], sreps[:, sl])
                for hh in range(2):
                    kv = 2 * m + hh
                    ko = slice(kv * S + sp * SPAN, kv * S + (sp + 1) * SPAN)
                    eng = nc.vector if hh == 0 else nc.gpsimd
                    eng.tensor_add(kT[:, ko], t1[hh * 64:hh * 64 + 64, :],
                                   t2[hh * 64:hh * 64 + 64, :])

            def s2_q(m):
                pq1 = bigp.tile([128, 1024], f32, tag="big", name="pq1")
                nc.tensor.matmul(pq1[:, 0:256], qbas[:, m * 128:(m + 1) * 128], cq[:],
                                 start=True, stop=True)
                nc.tensor.matmul(pq1[:, 512:768], qbss[:, m * 128:(m + 1) * 128], cq[:],
                                 start=True, stop=True)
                t1 = evp.tile([128, 256], f16, tag="t1q", name="t1q")
                t2 = evp.tile([128, 256], f16, tag="t2q", name="t2q")
                nc.vector.tensor_mul(t1[:], pq1[:, 0:256], crepqs[:])
                nc.vector.tensor_mul(t2[:], pq1[:, 512:768], srepqs[:])
                for hh in range(2):
                    h = 2 * m + hh
                    nc.vector.tensor_add(qT[:, h * 128:(h + 1) * 128],
                                         t1[hh * 64:hh * 64 + 64, 0:128],
                                         t2[hh * 64:hh * 64 + 64, 0:128])
                    nc.vector.tensor_add(qT[:, (H + h) * 128:(H + h + 1) * 128],
                                         t1[hh * 64:hh * 64 + 64, 128:256],
                                         t2[hh * 64:hh * 64 + 64, 128:256])

            VOFFS = (0, 130, 512, 642)

            def s2_v(m, t4):
                pv = bigp.tile([128, 1024], f32, tag="big", name="pv")
                for i in range(4):
                    t = t4 * 4 + i
                    nc.tensor.matmul(pv[:, VOFFS[i]:VOFFS[i] + 130],
                                     cv[:, t * 128:(t + 1) * 128],
                                     vbas[:, m * 130:(m + 1) * 130],
                                     start=True, stop=True)
                for i in range(4):
                    t = t4 * 4 + i
                    vo = (m * NB + t) * 130
                    src = pv[:, VOFFS[i]:VOFFS[i] + 130]
                    if i % 2 == 0:
                        nc.scalar.copy(vsb[:, vo:vo + 130], src)
                    else:
                        nc.vector.tensor_copy(vsb[:, vo:vo + 130], src)

            # interleave remaining stage-1 spans with stage-2 units so the
            # PE never drains while elementwise rope work catches up
            s2_k(0, 0)
            s2_k(1, 0)
            load_span(3)
            s1_kv(2)
            s2_v(0, 0)
            s2_v(1, 0)
            s2_k(0, 1)
            s2_k(1, 1)
            s2_q(0)
            s2_q(1)
            s1_kv(3)
            s2_k(0, 2)
            s2_k(1, 2)
            s2_v(0, 1)
            s2_v(1, 1)
            s2_q(2)
            s2_q(3)
            s2_k(0, 3)
            s2_k(1, 3)
            s2_v(0, 2)
            s2_v(1, 2)
            s2_q(4)
            s2_q(5)
            s2_v(0, 3)
            s2_v(1, 3)
            s2_q(6)
            s2_q(7)

            # ---------------- stage 3: attention (per-core ladder) ----------------
            def attn_body(c):
                blocks = [c, 15 - c]
                biL = 0 if blocks[0] >= blocks[1] else 1   # longer stream
                lenL = blocks[biL] + 1

                def slot(bi, t):
                    return t if bi == biL else lenL + t

                for kv in range(HKV):
                    m = kv // 2
                    voff = 65 * (kv % 2)
                    ko0 = kv * S
                    qo0 = 4 * kv * 128

                    pacc = [paccp.tile([65, 512], f32, tag="paccA", name="paccA"),
                            paccp.tile([65, 512], f32, tag="paccB", name="paccB")]
                    ptall = ptp.tile([128, 17 * 512], f16, tag="ptall", name="ptall")

                    def norm(bi, kv=kv):
                        # pacc[bi] row 64 holds the softmax denominator; scale
                        # rows 0:63 by its reciprocal and evict to onorm.
                        dens = evp.tile([1, 512], f32, tag="dens", name="dens")
                        nc.vector.tensor_copy(dens[:], pacc[bi][64:65, :])
                        recs = evp.tile([1, 512], f32, tag="recs", name="recs")
                        nc.vector.reciprocal_approx_fast(recs[:], dens[:])
                        rbs = evp.tile([64, 512], f32, tag="rbs", name="rbs")
                        nc.gpsimd.partition_broadcast(rbs[:], recs[:])
                        for hp in range(4):
                            h = 4 * kv + hp
                            mo = (h // 2) * 256 + bi * 128
                            nc.vector.tensor_mul(
                                onorm[64 * (hp % 2):64 * (hp % 2) + 64, mo:mo + 128],
                                pacc[bi][0:64, hp * 128:(hp + 1) * 128],
                                rbs[:, hp * 128:(hp + 1) * 128])

                    units = []
                    for bi, j in enumerate(blocks):
                        u = []
                        t = 0
                        while t <= j:
                            n = 2 if t + 1 <= j else 1
                            u.append((bi, t, n))
                            t += n
                        units.append(u)
                    order = []
                    a, b = (units[biL], units[1 - biL])
                    for i in range(max(len(a), len(b))):
                        if i < len(a):
                            order.append(a[i])
                        if i < len(b):
                            order.append(b[i])

                    pend_pv = []
                    for (bi, t0, n) in order:
                        j = blocks[bi]
                        sc = bigp.tile([128, 1024], f32, tag="big", name="sc")
                        for i in range(n):
                            t = t0 + i
                            nc.tensor.matmul(
                                sc[:, i * 512:(i + 1) * 512],
                                kT[:, ko0 + t * 128: ko0 + (t + 1) * 128],
                                qT[:, bi * H * 128 + qo0: bi * H * 128 + qo0 + 512],
                                start=True, stop=True)
                        if len(pend_pv) >= 3:
                            pend_pv.pop(0)()
                        s0 = slot(bi, t0)
                        nc.scalar.activation(ptall[:, s0 * 512:(s0 + n) * 512],
                                             sc[:, 0:n * 512], AF.Exp)
                        if t0 <= j <= t0 + n - 1:
                            sj = slot(bi, j)
                            nc.vector.tensor_mul(ptall[:, sj * 512:(sj + 1) * 512],
                                                 ptall[:, sj * 512:(sj + 1) * 512], tris[:])

                        def mk(bi=bi, t0=t0, n=n, j=j):
                            def emit():
                                for i in range(n):
                                    t = t0 + i
                                    st = slot(bi, t)
                                    nc.tensor.matmul(
                                        pacc[bi][0:65, :],
                                        vsb[:, (m * NB + t) * 130 + voff:
                                             (m * NB + t) * 130 + voff + 65],
                                        ptall[:, st * 512:(st + 1) * 512],
                                        start=(t == 0), stop=(t == j))
                                if t0 + n - 1 == j:
                                    norm(bi)   # stream done: drain + free pacc
                            return emit
                        pend_pv.append(mk())
                    for fn in pend_pv:
                        fn()

            for case in tc.Switch(pid, NCORES):
                attn_body(case)

            # ---------------- stage 4: output projection (uniform) ----------------
            for bi in range(2):
                for e2 in range(2):
                    py = bigp.tile([128, 1024], f32, tag="big", name="py")
                    for half in range(2):
                        e = e2 * 2 + half
                        for mm in range(8):
                            nc.tensor.matmul(
                                py[:, half * 512:(half + 1) * 512],
                                onorm[:, mm * 256 + bi * 128: mm * 256 + bi * 128 + 128],
                                wtes[e][:, mm * 512:(mm + 1) * 512],
                                start=(mm == 0), stop=(mm == 7))
                        ye = evp.tile([128, 512], f16, tag=f"ye{half}", name=f"ye{half}")
                        nc.scalar.copy(ye[:], py[:, half * 512:(half + 1) * 512])
                        nc.sync.dma_start(
                            out=yout[bi, :, e2 * 1024 + half * 512: e2 * 1024 + (half + 1) * 512],
                            in_=ye[:])

            if DBG:
                nc.sync.dma_start(out=dbg["cko"][:], in_=ck[:])
                nc.sync.dma_start(out=dbg["cvo"][:], in_=cv[:])
                nc.sync.dma_start(out=dbg["cqo"][:], in_=cq[:])
                nc.sync.dma_start(out=dbg["kTo"][:], in_=kT[:])
                nc.sync.dma_start(out=dbg["qTo"][:], in_=qT[:])
                nc.sync.dma_start(out=dbg["vsbo"][:], in_=vsb[:])
                nc.sync.dma_start(out=dbg["onormo"][:], in_=onorm[:])

    nc.finalize()
    return nc


def kernel(**inputs):
    if "nc" not in _CACHE:
        _CACHE["nc"] = _build_program()
    nc = _CACHE["nc"]
    from concourse.bass_utils import run_bass_kernel_spmd

    in_maps = _prep(inputs)
    res = run_bass_kernel_spmd(nc, in_maps, list(range(NCORES)))
    y = np.zeros((1, S, DIM), np.float32)
    for c in range(NCORES):
        yc = np.asarray(res.results[c]["y"], np.float32)
        y[0, c * 128:(c + 1) * 128] = yc[0]
        y[0, (15 - c) * 128:(16 - c) * 128] = yc[1]
    return y



# revision 65
# speedup vs baseline: 1.0105x; 1.0105x over previous
import sys, os

sys.path.insert(0, "/opt/trn_rl_repo")
sys.path.insert(0, "/root/.axon_site")
import numpy as np

DIM = 2048
DH = 64
H = 16
HKV = 4
G = H // HKV
RANK = 8
S = 2048
NCORES = 8
NB = S // 128   # 16 q-blocks of 128 rows
NSPAN = 4       # 4 spans of 512 over S
SPAN = 512
ND = DIM // 128  # 16 D-tiles

_CACHE = {}


def _deint_perm():
    # even dims 0,2,..62 -> rows 0..31 ; odd dims -> rows 32..63
    p = np.zeros(DH, np.int64)
    for i in range(DH // 2):
        p[i] = 2 * i
        p[32 + i] = 2 * i + 1
    return p


def _prep(inputs):
    """Host-side prep; returns per-core input maps (uniform shapes)."""
    f16 = np.float16
    x = np.asarray(inputs["x"], np.float32)[0]          # (S, D)
    xt = np.ascontiguousarray(x.T).astype(f16)          # (D, S)
    perm = _deint_perm()

    wq = np.asarray(inputs["wq"], np.float32)[perm] * 0.125   # (64, D) permuted + scale
    wk = np.asarray(inputs["wk"], np.float32)[perm]
    wv = np.asarray(inputs["wv"], np.float32)
    wq_a = np.asarray(inputs["wq_a"], np.float32)
    wk_a = np.asarray(inputs["wk_a"], np.float32)
    wv_a = np.asarray(inputs["wv_a"], np.float32)
    wq_b = np.asarray(inputs["wq_b"], np.float32).reshape(H, DH, RANK)[:, perm, :]
    wk_b = np.asarray(inputs["wk_b"], np.float32).reshape(HKV, DH, RANK)[:, perm, :]
    wv_b = np.asarray(inputs["wv_b"], np.float32).reshape(HKV, DH, RANK)

    w1t = np.ascontiguousarray(np.concatenate([wk, wv], 0).T).astype(f16)      # (D, 128)
    w2 = np.zeros((48, DIM), np.float32)   # 32-aligned: k_a@0:8, v_a@32:40
    w2[0:8] = wk_a
    w2[32:40] = wv_a
    w2t = np.ascontiguousarray(w2.T).astype(f16)                               # (D, 48)
    wqt = np.ascontiguousarray(np.concatenate([wq, wq_a], 0).T).astype(f16)    # (D, 72)

    def baug(wb, scale, swap):
        nh = wb.shape[0]
        out = np.zeros((nh // 2, 128, 128), np.float32)
        for m in range(nh // 2):
            for hh in range(2):
                h = 2 * m + hh
                for d in range(DH):
                    dd = (d + 32) % DH if swap else d
                    col = 64 * hh + d
                    out[m, dd, col] = 1.0
                    out[m, 64:72, col] = wb[h, dd] * scale
        return out.astype(f16)

    kba = baug(wk_b, 2.0, False)
    kbs = baug(wk_b, 2.0, True)
    qba = baug(wq_b, 0.25, False)
    qbs = baug(wq_b, 0.25, True)

    # v B-proj rhs with interleaved ones-columns:
    # per m chunk [128, 130] = [v(2m) 64 | one | v(2m+1) 64 | one]
    vba2 = np.zeros((128, 2 * 130), np.float32)
    for m in range(2):
        o = m * 130
        for hh in range(2):
            h = 2 * m + hh
            co = o + hh * 65
            for d in range(DH):
                vba2[d, co + d] = 1.0
                vba2[64:72, co + d] = wv_b[h, d] * 2.0
        vba2[96, o + 64] = 1.0
        vba2[96, o + 129] = 1.0
    vba2 = vba2.astype(f16)

    wo = np.asarray(inputs["wo"], np.float32)              # (D, 64)
    wo_share = np.asarray(inputs["wo_share"], np.float32)  # (D, 1024)
    wc = wo_share + np.tile(wo, (1, H))
    wct = np.ascontiguousarray(wc.T).astype(f16)           # (1024, D)

    fc = np.asarray(inputs["freq_cis"], np.float32)        # (S, 32, 2)
    cos = fc[:, :, 0].T                                    # (32, S)
    sin = fc[:, :, 1].T
    crep = np.tile(cos, (4, 1)).astype(f16)                # (128, S)
    sr = np.concatenate([-sin, sin], 0)                    # (64, S)
    srep = np.tile(sr, (2, 1)).astype(f16)                 # (128, S)

    tri = (np.arange(128)[:, None] <= np.arange(128)[None, :]).astype(f16)
    tri4 = np.ascontiguousarray(np.tile(tri, (1, 4)))      # (128, 512)
    ident = np.eye(128, dtype=f16)
    mask4 = np.ascontiguousarray((1.0 - tri4) * np.float16(-30000.0)).astype(f16)

    # pre-rearrange to the exact SBUF image [128, free] so every DMA is a
    # contiguous per-partition copy (strided gathers were ~3x slower)
    def sbimg(a2d, p=128):
        # (d p) f -> p (d f)
        D2, F = a2d.shape
        d = D2 // p
        return np.ascontiguousarray(a2d.reshape(d, p, F).transpose(1, 0, 2).reshape(p, d * F))

    xtsp = np.stack([sbimg(np.ascontiguousarray(xt[:, sp * 512:(sp + 1) * 512]))
                     for sp in range(4)])                      # (4, 128, 8192)
    w1c = sbimg(w1t)
    w2c = sbimg(w2t)
    wqc = sbimg(wqt)
    wcte = np.stack([sbimg(np.ascontiguousarray(wct[:, e * 512:(e + 1) * 512]))
                     for e in range(4)])                       # (4, 128, 4096)
    kbaf = np.ascontiguousarray(kba.transpose(1, 0, 2).reshape(128, 2 * 128))
    kbsf = np.ascontiguousarray(kbs.transpose(1, 0, 2).reshape(128, 2 * 128))
    qbaf = np.ascontiguousarray(qba.transpose(1, 0, 2).reshape(128, 8 * 128))
    qbsf = np.ascontiguousarray(qbs.transpose(1, 0, 2).reshape(128, 8 * 128))

    shared = dict(
        xtsp=xtsp, w1c=w1c, w2c=w2c, wqc=wqc,
        kbaf=kbaf, kbsf=kbsf, qbaf=qbaf, qbsf=qbsf, vba2=vba2,
        wcte=wcte, crep=crep, srep=srep, tri4=tri4,
    )

    per_core = []
    for c in range(NCORES):
        blocks = [c, 15 - c]
        cols = np.concatenate([np.arange(b * 128, (b + 1) * 128) for b in blocks])
        m = dict(shared)
        m.update(
            xqc=sbimg(np.ascontiguousarray(xt[:, cols])),
            crepq=np.ascontiguousarray(crep[:, cols]),
            srepq=np.ascontiguousarray(srep[:, cols]),
        )
        per_core.append(m)
    return per_core


def _build_program():
    import concourse.bass as bass
    import concourse.bacc as bacc
    import concourse.mybir as mybir
    from concourse import tile

    f16 = mybir.dt.float16
    f32 = mybir.dt.float32
    AF = mybir.ActivationFunctionType

    nc = bacc.Bacc("TRN2", target_bir_lowering=False)

    def inp(name, shape, dt=f16):
        return nc.dram_tensor(name, list(shape), dt, kind="ExternalInput")

    xtsp = inp("xtsp", (NSPAN, 128, ND * SPAN))
    w1c = inp("w1c", (128, ND * 128))
    w2c = inp("w2c", (128, ND * 48))
    wqc = inp("wqc", (128, ND * 72))
    kba = inp("kbaf", (128, 2 * 128))
    kbs = inp("kbsf", (128, 2 * 128))
    qba = inp("qbaf", (128, 8 * 128))
    qbs = inp("qbsf", (128, 8 * 128))
    vba2 = inp("vba2", (128, 260))
    wcte = inp("wcte", (4, 128, 8 * 512))
    crep = inp("crep", (128, S))
    srep = inp("srep", (128, S))
    tri4 = inp("tri4", (128, 512))
    xqc = inp("xqc", (128, ND * 256))
    crepq = inp("crepq", (128, 256))
    srepq = inp("srepq", (128, 256))

    yout = nc.dram_tensor("y", [2, 128, DIM], f16, kind="ExternalOutput")
    DBG = bool(os.environ.get("KDBG"))
    if DBG:
        dbg = {
            "cko": nc.dram_tensor("cko", [128, S], f16, kind="ExternalOutput"),
            "cvo": nc.dram_tensor("cvo", [128, S], f16, kind="ExternalOutput"),
            "cqo": nc.dram_tensor("cqo", [128, 256], f16, kind="ExternalOutput"),
            "kTo": nc.dram_tensor("kTo", [64, HKV * S], f16, kind="ExternalOutput"),
            "qTo": nc.dram_tensor("qTo", [64, 2 * H * 128], f16, kind="ExternalOutput"),
            "vsbo": nc.dram_tensor("vsbo", [128, 2 * NB * 130], f16, kind="ExternalOutput"),
            "onormo": nc.dram_tensor("onormo", [128, 8 * 256], f16, kind="ExternalOutput"),
        }

    pid = nc.partition_id()

    with tile.TileContext(nc) as tc:
        with (
            tc.tile_pool(name="const", bufs=1) as constp,
            tc.tile_pool(name="xts", bufs=2) as xtp,
            tc.tile_pool(name="pt", bufs=2) as ptp,
            tc.tile_pool(name="ev", bufs=2) as evp,
            tc.tile_pool(name="big", bufs=3, space="PSUM") as bigp,
            tc.tile_pool(name="pacc", bufs=1, space="PSUM") as paccp,
        ):
            # ---------------- persistent SBUF ----------------
            # DMA queue plan (engine queues serialize; spread + order by need):
            #  scalar: w1s, span1, wte0, wte1
            #  sync:   span0 (2 halves), span2, wte2, wte3
            #  vector: wqs, xqs, span3
            #  gpsimd: w2s, kba/kbs, creps/sreps, vba/qba/qbs, crepq/srepq, tris
            w1s = constp.tile([128, ND * 128], f16, tag="w1s", name="w1s")
            w2s = constp.tile([128, ND * 48], f16, tag="w2s", name="w2s")
            wqs = constp.tile([128, ND * 72], f16, tag="wqs", name="wqs")
            xqs = constp.tile([128, ND * 256], f16, tag="xqs", name="xqs")
            # priority load: the first weight the PE needs
            nc.scalar.dma_start(out=w1s[:], in_=w1c[:])

            kbas = constp.tile([128, 2 * 128], f16, tag="kbas", name="kbas")
            kbss = constp.tile([128, 2 * 128], f16, tag="kbss", name="kbss")
            creps = constp.tile([128, S], f16, tag="creps", name="creps")
            sreps = constp.tile([128, S], f16, tag="sreps", name="sreps")
            vbas = constp.tile([128, 2 * 130], f16, tag="vbas", name="vbas")
            qbas = constp.tile([128, 8 * 128], f16, tag="qbas", name="qbas")
            qbss = constp.tile([128, 8 * 128], f16, tag="qbss", name="qbss")
            crepqs = constp.tile([128, 256], f16, tag="crepqs", name="crepqs")
            srepqs = constp.tile([128, 256], f16, tag="srepqs", name="srepqs")
            tris = constp.tile([128, 512], f16, tag="tris", name="tris")

            wtes = [constp.tile([128, 8 * 512], f16, tag=f"wte{e}", name=f"wte{e}")
                    for e in range(4)]

            def issue_const_dmas():
                # gated behind stage-1 progress so these transfers don't
                # contend with the critical span-0/weight loads; ordered by
                # first use (stage-2 k, q, v consts, mask, out-proj weights)
                nc.gpsimd.dma_start(out=kbas[:], in_=kba[:])
                nc.gpsimd.dma_start(out=kbss[:], in_=kbs[:])
                nc.gpsimd.dma_start(out=vbas[:], in_=vba2[:])
                nc.gpsimd.dma_start(out=crepqs[:], in_=crepq[:])
                nc.gpsimd.dma_start(out=srepqs[:], in_=srepq[:])
                nc.gpsimd.dma_start(out=creps[:], in_=crep[:])
                nc.gpsimd.dma_start(out=sreps[:], in_=srep[:])
                nc.gpsimd.dma_start(out=qbas[:], in_=qba[:])
                nc.gpsimd.dma_start(out=qbss[:], in_=qbs[:])
                nc.gpsimd.dma_start(out=tris[:], in_=tri4[:])
                for e in range(4):
                    nc.gpsimd.dma_start(out=wtes[e][:], in_=wcte[e, :, :])

            ck = constp.tile([128, S], f16, tag="ck", name="ck")
            cv = constp.tile([128, S], f16, tag="cv", name="cv")
            cq = constp.tile([128, 256], f16, tag="cq", name="cq")
            kT = constp.tile([64, HKV * S], f16, tag="kT", name="kT")
            vsb = constp.tile([128, 2 * NB * 130], f16, tag="vsb", name="vsb")
            qT = constp.tile([64, 2 * H * 128], f16, tag="qT", name="qT")
            onorm = constp.tile([128, 8 * 256], f16, tag="onorm", name="onorm")

            # warm-up: keep the PE busy while the first loads land so the
            # HAM clock gate releases (4/8 -> 8/8) before real work starts
            dmy = constp.tile([128, 512], f16, tag="dmy", name="dmy")
            nc.vector.memset(dmy[:], 0.0)
            for _ in range(16):
                pw = bigp.tile([128, 1024], f32, tag="big", name="pw")
                nc.tensor.matmul(pw[:, 0:512], dmy[:, 0:128], dmy[:],
                                 start=True, stop=True)

            ones1 = constp.tile([1, 64], f16, tag="ones1", name="ones1")
            nc.vector.memset(ones1[:], 1.0)
            nc.vector.memset(ck[:], 0.0)
            nc.gpsimd.memset(cv[:], 0.0)
            nc.vector.memset(cq[:], 0.0)
            nc.gpsimd.memset(cv[96:97, :], 1.0)   # ones row for v denominator trick

            # ---------------- stage 1: projections (uniform) ----------------
            # span DMAs: issue all up front (split into halves for earlier
            # compute start), spread across scalar/sync/vector queues.
            span_eng = [nc.sync, nc.scalar, nc.sync, nc.scalar]
            xtas = [None] * NSPAN

            def load_span(sp):
                xta = xtp.tile([128, ND * SPAN], f16, tag="xta", name="xta")
                if sp == 0:
                    # race span 0 in on three queues at once
                    engs = [nc.sync, nc.sync, nc.scalar, nc.gpsimd]
                    for h in range(4):
                        dlo, dhi = h * 4, (h + 1) * 4
                        engs[h].dma_start(out=xta[:, dlo * SPAN:dhi * SPAN],
                                          in_=xtsp[sp, :, dlo * SPAN:dhi * SPAN])
                elif sp == 1:
                    nc.sync.dma_start(out=xta[:], in_=xtsp[sp, :, :])
                elif sp == 2:
                    nc.scalar.dma_start(out=xta[:], in_=xtsp[sp, :, :])
                else:
                    nc.scalar.dma_start(out=xta[:, 0:8 * SPAN], in_=xtsp[sp, :, 0:8 * SPAN])
                    nc.sync.dma_start(out=xta[:, 8 * SPAN:], in_=xtsp[sp, :, 8 * SPAN:])
                xtas[sp] = xta

            def s1_kv(sp):
                xta = xtas[sp]
                pkv = bigp.tile([128, 1024], f32, tag="big", name="pkv")
                for d in range(ND):
                    nc.tensor.matmul(pkv[:, 0:512], w1s[:, d * 128:(d + 1) * 128],
                                     xta[:, d * SPAN:(d + 1) * SPAN],
                                     start=(d == 0), stop=(d == ND - 1))
                for d in range(ND):
                    nc.tensor.matmul(pkv[0:48, 512:1024], w2s[:, d * 48:(d + 1) * 48],
                                     xta[:, d * SPAN:(d + 1) * SPAN],
                                     start=(d == 0), stop=(d == ND - 1))
                sl = slice(sp * SPAN, (sp + 1) * SPAN)
                nc.vector.tensor_copy(ck[0:64, sl], pkv[0:64, 0:512])
                nc.vector.tensor_copy(ck[64:72, sl], pkv[0:8, 512:1024])
                nc.scalar.copy(cv[0:64, sl], pkv[64:128, 0:512])
                nc.vector.tensor_copy(cv[64:72, sl], pkv[32:40, 512:1024])

            load_span(0)
            # rest of the near-term weights, after span 0 is in flight
            nc.scalar.dma_start(out=w2s[:], in_=w2c[:])
            nc.gpsimd.dma_start(out=wqs[:], in_=wqc[:])
            nc.gpsimd.dma_start(out=xqs[:], in_=xqc[:])
            load_span(1)
            s1_kv(0)
            # gate the bulk const loads behind span-0 eviction so their
            # transfers don't steal HBM bandwidth from the critical path
            gatet = constp.tile([1, 1], f16, tag="gatet", name="gatet")
            nc.gpsimd.tensor_copy(gatet[:], ck[0:1, 0:1])
            issue_const_dmas()
            load_span(2)
            s1_kv(1)
            pqt = bigp.tile([128, 1024], f32, tag="big", name="pqt")
            for d in range(ND):
                nc.tensor.matmul(pqt[0:72, 0:256], wqs[:, d * 72:(d + 1) * 72],
                                 xqs[:, d * 256:(d + 1) * 256],
                                 start=(d == 0), stop=(d == ND - 1))
            nc.vector.tensor_copy(cq[0:72, :], pqt[0:72, 0:256])
            # (spans 2/3 compute is interleaved with stage 2 below)

            # ---------------- stage 2: B-projections + rope (uniform) ----------------
            def s2_k(m, sp):
                sl = slice(sp * SPAN, (sp + 1) * SPAN)
                pk = bigp.tile([128, 1024], f32, tag="big", name="pk")
                nc.tensor.matmul(pk[:, 0:512], kbas[:, m * 128:(m + 1) * 128], ck[:, sl],
                                 start=True, stop=True)
                nc.tensor.matmul(pk[:, 512:1024], kbss[:, m * 128:(m + 1) * 128], ck[:, sl],
                                 start=True, stop=True)
                t1 = evp.tile([128, 512], f16, tag="t1", name="t1")
                t2 = evp.tile([128, 512], f16, tag="t2", name="t2")
                nc.vector.tensor_mul(t1[:], pk[:, 0:512], creps[:, sl])
                nc.vector.tensor_mul(t2[:], pk[:, 512:1024], sreps[:, sl])
                for hh in range(2):
                    kv = 2 * m + hh
                    ko = slice(kv * S + sp * SPAN, kv * S + (sp + 1) * SPAN)
                    nc.gpsimd.tensor_add(kT[:, ko], t1[hh * 64:hh * 64 + 64, :],
                                         t2[hh * 64:hh * 64 + 64, :])

            def s2_q(m):
                pq1 = bigp.tile([128, 1024], f32, tag="big", name="pq1")
                nc.tensor.matmul(pq1[:, 0:256], qbas[:, m * 128:(m + 1) * 128], cq[:],
                                 start=True, stop=True)
                nc.tensor.matmul(pq1[:, 512:768], qbss[:, m * 128:(m + 1) * 128], cq[:],
                                 start=True, stop=True)
                t1 = evp.tile([128, 256], f16, tag="t1q", name="t1q")
                t2 = evp.tile([128, 256], f16, tag="t2q", name="t2q")
                nc.vector.tensor_mul(t1[:], pq1[:, 0:256], crepqs[:])
                nc.vector.tensor_mul(t2[:], pq1[:, 512:768], srepqs[:])
                for hh in range(2):
                    h = 2 * m + hh
                    nc.vector.tensor_add(qT[:, h * 128:(h + 1) * 128],
                                         t1[hh * 64:hh * 64 + 64, 0:128],
                                         t2[hh * 64:hh * 64 + 64, 0:128])
                    nc.vector.tensor_add(qT[:, (H + h) * 128:(H + h + 1) * 128],
                                         t1[hh * 64:hh * 64 + 64, 128:256],
                                         t2[hh * 64:hh * 64 + 64, 128:256])

            VOFFS = (0, 130, 512, 642)

            def s2_v(m, t4):
                pv = bigp.tile([128, 1024], f32, tag="big", name="pv")
                for i in range(4):
                    t = t4 * 4 + i
                    nc.tensor.matmul(pv[:, VOFFS[i]:VOFFS[i] + 130],
                                     cv[:, t * 128:(t + 1) * 128],
                                     vbas[:, m * 130:(m + 1) * 130],
                                     start=True, stop=True)
                for i in range(4):
                    t = t4 * 4 + i
                    vo = (m * NB + t) * 130
                    src = pv[:, VOFFS[i]:VOFFS[i] + 130]
                    if i % 2 == 0:
                        nc.scalar.copy(vsb[:, vo:vo + 130], src)
                    else:
                        nc.vector.tensor_copy(vsb[:, vo:vo + 130], src)

            # interleave remaining stage-1 spans with stage-2 units so the
            # PE never drains while elementwise rope work catches up
            s2_k(0, 0)
            s2_k(1, 0)
            load_span(3)
            s1_kv(2)
            s2_v(0, 0)
            s2_v(1, 0)
            s2_k(0, 1)
            s2_k(1, 1)
            s2_q(0)
            s2_q(1)
            s1_kv(3)
            s2_k(0, 2)
            s2_k(1, 2)
            s2_v(0, 1)
            s2_v(1, 1)
            s2_q(2)
            s2_q(3)
            s2_k(0, 3)
            s2_k(1, 3)
            s2_v(0, 2)
            s2_v(1, 2)
            s2_q(4)
            s2_q(5)
            s2_v(0, 3)
            s2_v(1, 3)
            s2_q(6)
            s2_q(7)

            # ---------------- stage 3: attention (per-core ladder) ----------------
            def attn_body(c):
                blocks = [c, 15 - c]
                biL = 0 if blocks[0] >= blocks[1] else 1   # longer stream
                lenL = blocks[biL] + 1

                def slot(bi, t):
                    return t if bi == biL else lenL + t

                for kv in range(HKV):
                    m = kv // 2
                    voff = 65 * (kv % 2)
                    ko0 = kv * S
                    qo0 = 4 * kv * 128

                    pacc = [paccp.tile([65, 512], f32, tag="paccA", name="paccA"),
                            paccp.tile([65, 512], f32, tag="paccB", name="paccB")]
                    ptall = ptp.tile([128, 17 * 512], f16, tag="ptall", name="ptall")

                    def norm(bi, kv=kv):
                        # pacc[bi] row 64 holds the softmax denominator; scale
                        # rows 0:63 by its reciprocal and evict to onorm.
                        dens = evp.tile([1, 512], f32, tag="dens", name="dens")
                        nc.vector.tensor_copy(dens[:], pacc[bi][64:65, :])
                        recs = evp.tile([1, 512], f32, tag="recs", name="recs")
                        nc.vector.reciprocal_approx_fast(recs[:], dens[:])
                        rbs = evp.tile([64, 512], f32, tag="rbs", name="rbs")
                        nc.gpsimd.partition_broadcast(rbs[:], recs[:])
                        for hp in range(4):
                            h = 4 * kv + hp
                            mo = (h // 2) * 256 + bi * 128
                            nc.vector.tensor_mul(
                                onorm[64 * (hp % 2):64 * (hp % 2) + 64, mo:mo + 128],
                                pacc[bi][0:64, hp * 128:(hp + 1) * 128],
                                rbs[:, hp * 128:(hp + 1) * 128])

                    units = []
                    for bi, j in enumerate(blocks):
                        u = []
                        t = 0
                        while t <= j:
                            n = 2 if t + 1 <= j else 1
                            u.append((bi, t, n))
                            t += n
                        units.append(u)
                    order = []
                    a, b = (units[biL], units[1 - biL])
                    for i in range(max(len(a), len(b))):
                        if i < len(a):
                            order.append(a[i])
                        if i < len(b):
                            order.append(b[i])

                    pend_pv = []
                    for (bi, t0, n) in order:
                        j = blocks[bi]
                        sc = bigp.tile([128, 1024], f32, tag="big", name="sc")
                        for i in range(n):
                            t = t0 + i
                            nc.tensor.matmul(
                                sc[:, i * 512:(i + 1) * 512],
                                kT[:, ko0 + t * 128: ko0 + (t + 1) * 128],
                                qT[:, bi * H * 128 + qo0: bi * H * 128 + qo0 + 512],
                                start=True, stop=True)
                        if len(pend_pv) >= 3:
                            pend_pv.pop(0)()
                        s0 = slot(bi, t0)
                        nc.scalar.activation(ptall[:, s0 * 512:(s0 + n) * 512],
                                             sc[:, 0:n * 512], AF.Exp)
                        if t0 <= j <= t0 + n - 1:
                            sj = slot(bi, j)
                            nc.vector.tensor_mul(ptall[:, sj * 512:(sj + 1) * 512],
                                                 ptall[:, sj * 512:(sj + 1) * 512], tris[:])

                        def mk(bi=bi, t0=t0, n=n, j=j):
                            def emit():
                                for i in range(n):
                                    t = t0 + i
                                    st = slot(bi, t)
                                    nc.tensor.matmul(
                                        pacc[bi][0:65, :],
                                        vsb[:, (m * NB + t) * 130 + voff:
                                             (m * NB + t) * 130 + voff + 65],
                                        ptall[:, st * 512:(st + 1) * 512],
                                        start=(t == 0), stop=(t == j))
                                if t0 + n - 1 == j:
                                    norm(bi)   # stream done: drain + free pacc
                            return emit
                        pend_pv.append(mk())
                    for fn in pend_pv:
                        fn()

            for case in tc.Switch(pid, NCORES):
                attn_body(case)

            # ---------------- stage 4: output projection (uniform) ----------------
            for bi in range(2):
                for e2 in range(2):
                    py = bigp.tile([128, 1024], f32, tag="big", name="py")
                    for half in range(2):
                        e = e2 * 2 + half
                        for mm in range(8):
                            nc.tensor.matmul(
                                py[:, half * 512:(half + 1) * 512],
                                onorm[:, mm * 256 + bi * 128: mm * 256 + bi * 128 + 128],
                                wtes[e][:, mm * 512:(mm + 1) * 512],
                                start=(mm == 0), stop=(mm == 7))
                        ye = evp.tile([128, 512], f16, tag=f"ye{half}", name=f"ye{half}")
                        nc.scalar.copy(ye[:], py[:, half * 512:(half + 1) * 512])
                        nc.sync.dma_start(
                            out=yout[bi, :, e2 * 1024 + half * 512: e2 * 1024 + (half + 1) * 512],
                            in_=ye[:])

            if DBG:
                nc.sync.dma_start(out=dbg["cko"][:], in_=ck[:])
                nc.sync.dma_start(out=dbg["cvo"][:], in_=cv[:])
                nc.sync.dma_start(out=dbg["cqo"][:], in_=cq[:])
                nc.sync.dma_start(out=dbg["kTo"][:], in_=kT[:])
                nc.sync.dma_start(out=dbg["qTo"][:], in_=qT[:])
                nc.sync.dma_start(out=dbg["vsbo"][:], in_=vsb[:])
                nc.sync.dma_start(out=dbg["onormo"][:], in_=onorm[:])

    nc.finalize()
    return nc


def kernel(**inputs):
    if "nc" not in _CACHE:
        _CACHE["nc"] = _build_program()
    nc = _CACHE["nc"]
    from concourse.bass_utils import run_bass_kernel_spmd

    in_maps = _prep(inputs)
    res = run_bass_kernel_spmd(nc, in_maps, list(range(NCORES)))
    y = np.zeros((1, S, DIM), np.float32)
    for c in range(NCORES):
        yc = np.asarray(res.results[c]["y"], np.float32)
        y[0, c * 128:(c + 1) * 128] = yc[0]
        y[0, (15 - c) * 128:(16 - c) * 128] = yc[1]
    return y

